# revision 29
# baseline (speedup 1.0000x reference)
"""Cross-attention kernel for Trainium2, sharded over 8 NeuronCores.

Sharding: rows of (B, S1) split 8 ways -> each core handles one batch's
half (2048 query rows) and recomputes that batch's small KV projection.
No collectives needed.

Host-side preprocessing (free - doesn't count toward HW time):
  - transpose x, y to feature-major, pad with a ones-row so the PE adds bq
  - transpose weights; per-head deinterleave permutation of the head_dim
    axis on the Q/K sides turns the reference's interleaved rotate_half
    into contiguous-half rotation
  - fold qn_w, kn_w and the attention scale into cos/sin tables / LN rstd
  - kn_b is dropped entirely: it shifts all scores of a row equally, which
    softmax cancels.

v2 device pipeline (vs v1): grouped LN stats (one bn_stats per 4-head
group + even/odd combine), fused scalar_tensor_tensor centering,
half-width broadcast RoPE, single grouped softmax normalize, and a
software-pipelined emission order (scores lag Q-proj by 2 chunks, PV/out
lag by 4-5) so the PE queue never head-of-line blocks on the
LN -> RoPE -> DMA-transpose chains.
"""
import sys

sys.path.insert(0, '/opt/trn_rl_repo')

import numpy as np
import ml_dtypes

import concourse.bass as bass
import concourse.tile as tile
from concourse import bacc, mybir
from concourse.bass_utils import run_bass_kernel_spmd

bf16 = mybir.dt.bfloat16
f32 = mybir.dt.float32
AL = mybir.AluOpType
AF = mybir.ActivationFunctionType

# problem shapes (hardcoded per contest rules)
B, S1, S2, CQ, CKV, H, D = 4, 4096, 256, 1408, 1024, 16, 88
NCORES = 8
S = (B * S1) // NCORES          # 2048 query rows per core
NS = S // 128                   # 16 s-chunks
DP = 128                        # head_dim padded for transposes
G = 4                           # heads per LN group (4*88 = 352 cols)
NG = H // G
KC_Q = CQ // 128 + 1            # 12 contraction chunks (incl. bias ones-row)
KC_KV = CKV // 128              # 8
KC_O = CQ // 128                # 11
EPS = 1e-6
HALF = D // 2                   # 44
SCALE = float(D) ** -0.5

_BUILD_CACHE = {}


def _build(use_badd: bool, reps: int = 1):
    nc = bacc.Bacc("TRN2", target_bir_lowering=False)

    # const APs for the ACT sqrt biases (only 0.0/1.0 come pre-registered)
    for _v in (EPS * float(D), EPS):
        _t = nc.alloc_sbuf_tensor(f"const-f32-{_v}", [128, 1], f32)
        nc.gpsimd.memset(_t.ap(), _v)
        nc.const_aps.aps[(f32, _v)] = _t.ap()
    nc.all_engine_barrier()

    xT = nc.dram_tensor("xT", [128 * KC_Q, S], bf16, kind="ExternalInput")
    yT = nc.dram_tensor("yT", [CKV, S2], bf16, kind="ExternalInput")
    wq = nc.dram_tensor("wq", [128 * KC_Q, CQ], bf16, kind="ExternalInput")
    wkv = nc.dram_tensor("wkv", [CKV, 2 * CQ], bf16, kind="ExternalInput")
    wout = nc.dram_tensor("wout", [CQ, CQ], bf16, kind="ExternalInput")
    bkv = nc.dram_tensor("bkv", [2 * CQ], bf16, kind="ExternalInput")
    bout = nc.dram_tensor("bout", [CQ], bf16, kind="ExternalInput")
    # cos/sin folded tables, interleaved per chunk: [S, 2, DP]
    csw = nc.dram_tensor("csw", [S, 2, DP], bf16, kind="ExternalInput")
    if use_badd:
        badd = nc.dram_tensor("badd", [S, DP], f32, kind="ExternalInput")
    out = nc.dram_tensor("out", [S, CQ], bf16, kind="ExternalOutput")

    k_tiles = [(g * 352, 352) for g in range(NG)]
    v_tiles = [(CQ, 512), (CQ + 512, 512), (CQ + 1024, 384)]
    o_tiles = [(0, 512), (512, 512), (1024, 384)]

    with tile.TileContext(nc) as tc:
        with (
            tc.tile_pool(name="persist", bufs=1) as persist,
            tc.tile_pool(name="xq", bufs=2) as xqp,
            tc.tile_pool(name="cs", bufs=2) as csp,
            tc.tile_pool(name="qc", bufs=2) as qcp,
            tc.tile_pool(name="tt", bufs=2) as ttp,
            tc.tile_pool(name="stats", bufs=3) as statsp,
            tc.tile_pool(name="qropeT", bufs=3) as qropeTp,
            tc.tile_pool(name="attn", bufs=2) as attnp,
            tc.tile_pool(name="attnT", bufs=4) as attnTp,
            tc.tile_pool(name="cbf", bufs=3) as cbfp,
            tc.tile_pool(name="ctxT", bufs=1) as ctxTp,
            tc.tile_pool(name="outsb", bufs=2) as outsbp,
            tc.tile_pool(name="ps_big", bufs=3, space="PSUM") as ps_big,
            tc.tile_pool(name="ps_sc", bufs=3, space="PSUM") as ps_sc,
            tc.tile_pool(name="ps_ctx", bufs=2, space="PSUM") as ps_ctx,
        ):
            # ---------- persistent tiles ----------
            wq_sb = persist.tile([128, KC_Q, CQ], bf16, tag="wq_sb")

            def load_wq():
                _wq_engs = [nc.sync, nc.scalar, nc.sync, nc.scalar]
                for _g in range(NG):
                    _wq_engs[_g].dma_start(
                        wq_sb[:, :, _g * 352:(_g + 1) * 352],
                        wq[:].rearrange("(k p) o -> p k o", p=128)
                        [:, :, _g * 352:(_g + 1) * 352])
            wout_sb = persist.tile([128, KC_O, CQ], bf16, tag="wout_sb")
            yT_sb = persist.tile([128, KC_KV, S2], bf16, tag="yT_sb")
            bkv_ap = bkv[:]
            bkv_bc = persist.tile([128, 2 * CQ], bf16, tag="bkv_bc")

            def load_kv_consts():
                nc.gpsimd.dma_start(
                    yT_sb[:], yT[:].rearrange("(k p) t -> p k t", p=128))
                nc.gpsimd.dma_start(bkv_bc[:], bass.AP(
                    tensor=bkv_ap.tensor, offset=bkv_ap.offset,
                    ap=[[0, 128], *bkv_ap.ap]))
            bout_ap = bout[:]
            bout_bc = persist.tile([128, CQ], bf16, tag="bout_bc")

            kln = [persist.tile([128, H, DP], bf16, tag=f"kln{t}", name=f"kln{t}")
                   for t in range(2)]
            # kT layout: [d_pad, head, t]
            kT = persist.tile([128, H, S2], bf16, tag="kT")
            v_sb = persist.tile([128, 2, CQ], bf16, tag="v_sb")
            # q after RoPE, bf16, padded head_dim; persistent double buffer
            qrope = persist.tile([128, 2, H, DP], bf16, tag="qrope")

            def emit_ln_stats(st, nheads, dst_mu, dst_rd, sqrt_scale,
                              sqrt_bias, eng=None):
                """st: [128, nheads, 6] bn_stats output (even/odd half stats).
                dst_mu <- per-head mean; dst_rd <- 1/sqrt(var88*scale + bias)."""
                eng = eng or nc.vector
                # mu = 0.5*(m_e + m_o)
                t_s = statsp.tile([128, nheads], f32, tag="kt_s")
                eng.tensor_tensor(t_s[:], st[:, :, 1], st[:, :, 4], AL.add)
                eng.tensor_scalar_mul(out=dst_mu[:], in0=t_s[:], scalar1=0.5)
                # var*88 = M2_e + M2_o + 22*(m_e - m_o)^2
                t_d = statsp.tile([128, nheads], f32, tag="kt_d")
                eng.tensor_tensor(t_d[:], st[:, :, 1], st[:, :, 4], AL.subtract)
                eng.tensor_mul(t_d[:], t_d[:], t_d[:])
                t_m = statsp.tile([128, nheads], f32, tag="kt_m")
                eng.tensor_tensor(t_m[:], st[:, :, 2], st[:, :, 5], AL.add)
                eng.scalar_tensor_tensor(
                    out=t_m[:], in0=t_d[:], scalar=22.0, in1=t_m[:],
                    op0=AL.mult, op1=AL.add)
                # 1/sqrt(x) = exp(-0.5*ln(x)): Ln/Exp share one ACT table
                # with the softmax Exp, so no table reloads anywhere.
                nc.scalar.activation(out=t_m[:], in_=t_m[:], func=AF.Ln,
                                     scale=sqrt_scale, bias=sqrt_bias)
                nc.scalar.activation(out=dst_rd[:], in_=t_m[:], func=AF.Exp,
                                     scale=-0.5)

            def emit_body(rep):
                if rep == 0:
                    # zero the head_dim padding read by the qropeT transposes
                    nc.gpsimd.memset(qrope[:, :, :, D:DP], 0.0)
                # ---------- loads ----------
                xq_t = {}
                csw_t = {}
                badd_t = {}

                def emit_load(si):
                    xq_t[si] = xqp.tile([128, KC_Q, 128], bf16, tag="xq", name="xq")
                    nc.sync.dma_start(
                        xq_t[si][:], xT[:].rearrange("(k p) s -> p k s", p=128)
                        [:, :, si * 128:(si + 1) * 128])
                    csw_t[si] = csp.tile([128, 2, DP], bf16, tag="cs", name="cs")
                    nc.sync.dma_start(
                        csw_t[si][:], csw[si * 128:(si + 1) * 128, :, :])
                    if use_badd:
                        badd_t[si] = csp.tile([128, DP], f32, tag="ba", name="ba")
                        nc.sync.dma_start(
                            badd_t[si][:], badd[si * 128:(si + 1) * 128, :])

                qc_t = {}
                rd_q = {}

                def qproj_tiles(si):
                    """4 PE closures: 12 matmuls + grouped stats + centering."""
                    qc = qcp.tile([128, H, D], bf16, tag="qc", name="qc")
                    qc_t[si] = qc
                    st = statsp.tile([128, H, 6], f32, tag="st_q", name="st_q")
                    xq = xq_t.pop(si)

                    def group(g):
                        ps = ps_big.tile([128, 512], f32, tag="big", name="psq")
                        for kc in range(KC_Q):
                            nc.tensor.matmul(
                                ps[:, :352],
                                xq[:, kc, :],
                                wq_sb[:, kc, g * 352:(g + 1) * 352],
                                start=(kc == 0), stop=(kc == KC_Q - 1))
                        psv = ps[:, :352].rearrange("p (g d) -> p g d", d=D)
                        for g2 in range(G):
                            nc.vector.bn_stats(st[:, g * G + g2, :],
                                               psv[:, g2, :])
                        ssum = statsp.tile([128, G], f32, tag="ssum", name="ssum")
                        nc.vector.tensor_tensor(
                            ssum[:], st[:, g * G:(g + 1) * G, 1],
                            st[:, g * G:(g + 1) * G, 4], AL.add)
                        nc.vector.scalar_tensor_tensor(
                            out=qc[:, g * G:(g + 1) * G, :],
                            in0=ssum[:, :, None].to_broadcast([128, G, D]),
                            scalar=-0.5, in1=psv, op0=AL.mult, op1=AL.add)

                    def finish():
                        # rd = 1/sqrt(var88 + eps*D) = rstd/sqrt(D): the folded
                        # exp scale (stats count == D).  1/sqrt via
                        # exp(-0.5*ln(x)) keeps ACT on one table set.
                        t_d = statsp.tile([128, H], f32, tag="t_d", name="t_d")
                        nc.vector.tensor_tensor(t_d[:], st[:, :, 1],
                                                st[:, :, 4], AL.subtract)
                        nc.vector.tensor_mul(t_d[:], t_d[:], t_d[:])
                        t_m = statsp.tile([128, H], f32, tag="t_m", name="t_m")
                        nc.vector.tensor_tensor(t_m[:], st[:, :, 2],
                                                st[:, :, 5], AL.add)
                        nc.vector.scalar_tensor_tensor(
                            out=t_m[:], in0=t_d[:], scalar=22.0, in1=t_m[:],
                            op0=AL.mult, op1=AL.add)
                        rd = statsp.tile([128, H], f32, tag="rd_q", name="rd_q")
                        rd_q[si] = rd
                        nc.scalar.activation(out=t_m[:], in_=t_m[:], func=AF.Ln,
                                             scale=1.0, bias=EPS * float(D))
                        nc.scalar.activation(out=rd[:], in_=t_m[:], func=AF.Exp,
                                             scale=-0.5)

                    return [lambda g=g: group(g) for g in range(NG)], finish

                qropeT_t = {}
                attn_t = {}
                aT_t = {}

                def emit_qpost(si):
                    """RoPE (bf16 2x halves) + qropeT transposes."""
                    qc = qc_t.pop(si)
                    cswt = csw_t.pop(si)
                    qr = qrope[:, si % 2]
                    qT = qropeTp.tile([128, H, 128], bf16, tag="qropeT",
                                      name="qropeT")
                    qropeT_t[si] = qT
                    if use_badd:
                        nc.vector.tensor_tensor(
                            qc[:], qc[:],
                            rd_q[si][:, :, None].to_broadcast([128, H, D]),
                            AL.mult)
                    for hh in range(2):      # 8-head halves
                        hs = slice(hh * 8, (hh + 1) * 8)
                        t_t = ttp.tile([128, 8, D], bf16, tag="tt", name="tt")
                        nc.vector.tensor_mul(
                            t_t[:, :, 0:HALF], qc[:, hs, HALF:D],
                            cswt[:, 1, None, 0:HALF].to_broadcast([128, 8, HALF]))
                        nc.vector.tensor_mul(
                            t_t[:, :, HALF:D], qc[:, hs, 0:HALF],
                            cswt[:, 1, None, HALF:D].to_broadcast([128, 8, HALF]))
                        nc.vector.tensor_mul(
                            qc[:, hs, :], qc[:, hs, :],
                            cswt[:, 0, None, 0:D].to_broadcast([128, 8, D]))
                        nc.vector.tensor_tensor(
                            qr[:, hs, 0:D], qc[:, hs, :], t_t[:], AL.add)
                        if use_badd:
                            nc.vector.tensor_tensor(
                                qr[:, hs, 0:D], qr[:, hs, 0:D],
                                badd_t[si][:, None, 0:D].to_broadcast([128, 8, D]),
                                AL.add)
                        nc.sync.dma_start_transpose(
                            qT[:, hs, :],
                            qr[:, hs, :].rearrange("p h d -> p (h d)"))
                    if use_badd:
                        badd_t.pop(si)
                        rd_q.pop(si)

                def scores_tiles(si):
                    """8 PE closures (2 mm + 2 exp each) + finish (normalize)."""
                    qT = qropeT_t.pop(si)
                    attn = attnp.tile([128, H, S2], bf16, tag="attn", name="attn")
                    attn_t[si] = attn
                    denom = statsp.tile([128, H], f32, tag="denom", name="denom")
                    rdq = rd_q.pop(si) if not use_badd else None

                    def tile_fn(hp):
                        sps = ps_sc.tile([128, 2, S2], f32, tag="sc", name="sc")
                        for i in range(2):
                            h = 2 * hp + i
                            nc.tensor.matmul(sps[:, i, :], qT[:, h, :],
                                             kT[:, h, :], start=True, stop=True)
                            nc.scalar.activation(
                                out=attn[:, h, :], in_=sps[:, i, :],
                                func=AF.Exp,
                                scale=rdq[:, h:h + 1] if rdq is not None else SCALE,
                                accum_out=denom[:, h:h + 1])

                    def finish():
                        rd = statsp.tile([128, H], f32, tag="rd_sc", name="rd_sc")
                        nc.vector.reciprocal(rd[:], denom[:])
                        for h in range(H):
                            eng = nc.gpsimd if h % 2 == 0 else nc.vector
                            eng.tensor_scalar_mul(
                                out=attn[:, h, :], in0=attn[:, h, :],
                                scalar1=rd[:, h:h + 1])
                        # aT transposes
                        aT = attnTp.tile([128, 2 * H, 128], bf16, tag="attnT",
                                         name="attnT")
                        aT_t[si] = aT
                        for hh in range(2):
                            h0 = hh * 8
                            nc.sync.dma_start_transpose(
                                aT[:, 2 * h0:2 * h0 + 16, :],
                                attn[:, h0:h0 + 8, :]
                                .rearrange("p h t -> p (h t)"))
                        attn_t.pop(si)

                    return [lambda hp=hp: tile_fn(hp) for hp in range(H // 2)], \
                        finish

                ctxT_t = {}

                def pv_tiles(sj0):
                    """8 PE closures: PV for 2 heads + evac + ctxT scatter."""
                    aTs = [aT_t.pop(sj0), aT_t.pop(sj0 + 1)]
                    ctxT = ctxTp.tile([128, KC_O, 256], bf16, tag="ctxT",
                                      name="ctxT")
                    ctxT_t[sj0] = ctxT
                    dma_engines = [nc.scalar, nc.gpsimd]

                    def tile_fn(hp):
                        cps = ps_ctx.tile([D, 2, 256], f32, tag="cps", name="cps")
                        for i in range(2):
                            h = 2 * hp + i
                            for s2 in range(2):
                                for t in range(2):
                                    nc.tensor.matmul(
                                        cps[:, i, s2 * 128:(s2 + 1) * 128],
                                        v_sb[:, t, h * D:(h + 1) * D],
                                        aTs[s2][:, 2 * h + t, :],
                                        start=(t == 0), stop=(t == 1))
                        cbf = cbfp.tile([D, 2, 256], bf16, tag="cbf", name="cbf")
                        if hp % 2 == 0:
                            nc.vector.tensor_copy(cbf[:], cps[:])
                        else:
                            nc.scalar.copy(cbf[:], cps[:])
                        for i in range(2):
                            h = 2 * hp + i
                            c0 = h * D
                            r0, ch0 = c0 % 128, c0 // 128
                            n1 = min(128 - r0, D)
                            eng = dma_engines[(2 * hp + i) % 2]
                            eng.dma_start(ctxT[r0:r0 + n1, ch0, :],
                                          cbf[0:n1, i, :])
                            if n1 < D:
                                eng.dma_start(ctxT[0:D - n1, ch0 + 1, :],
                                              cbf[n1:D, i, :])

                    return [lambda hp=hp: tile_fn(hp) for hp in range(H // 2)]

                def op_tiles(sj0):
                    """6 PE closures: out-proj o-tile for one s2 + bias; the
                    out DMAs are deferred to op_finish so sync-queue
                    transposes aren't stuck behind them."""
                    ctxT = ctxT_t.pop(sj0)
                    pending = []

                    def tile_fn(s2, o0, ow):
                        sj = sj0 + s2
                        pso = ps_big.tile([128, 512], f32, tag="big", name="pso")
                        for c in range(KC_O):
                            nc.tensor.matmul(
                                pso[:, :ow],
                                ctxT[:, c, s2 * 128:(s2 + 1) * 128],
                                wout_sb[:, c, o0:o0 + ow],
                                start=(c == 0), stop=(c == KC_O - 1))
                        osb = outsbp.tile([128, 512], bf16, tag="outsb",
                                          name="osb")
                        nc.vector.tensor_tensor(
                            osb[:, :ow], pso[:, :ow],
                            bout_bc[:, o0:o0 + ow], AL.add)
                        pending.append((sj, o0, ow, osb))

                    def finish():
                        for (sj, o0, ow, osb) in pending:
                            nc.sync.dma_start(
                                out[sj * 128:(sj + 1) * 128, o0:o0 + ow],
                                osb[:, :ow])
                        pending.clear()

                    return [lambda s2=s2, o0=o0, ow=ow: tile_fn(s2, o0, ow)
                            for s2 in range(2) for (o0, ow) in o_tiles], finish

                kv_pending = []

                def prefetch_kv(n):
                    tiles = k_tiles + v_tiles
                    while len(kv_pending) < len(tiles) and n > 0:
                        wi = len(kv_pending)
                        o0, ow = tiles[wi]
                        wkv_t = attnp.tile([128, KC_KV, ow], bf16, tag="attn",
                                           name="wkv_t")
                        (nc.sync if wi % 2 == 0 else nc.scalar).dma_start(
                            wkv_t[:],
                            wkv[:].rearrange("(k p) o -> p k o", p=128)
                            [:, :, o0:o0 + ow])
                        kv_pending.append(wkv_t)
                        n -= 1

                def kv_tile(wi):
                    o0, ow = (k_tiles + v_tiles)[wi]
                    prefetch_kv(wi + 2 - len(kv_pending))
                    wkv_t = kv_pending[wi]
                    for t in range(2):
                        ps = ps_big.tile([128, 512], f32, tag="big")
                        for kc in range(KC_KV):
                            nc.tensor.matmul(
                                ps[:, :ow],
                                yT_sb[:, kc, t * 128:(t + 1) * 128],
                                wkv_t[:, kc, :],
                                start=(kc == 0), stop=(kc == KC_KV - 1))
                        if o0 < CQ:
                            g0 = o0 // 352 * G
                            kb = ttp.tile([128, G, D], f32, tag="kb")
                            nc.vector.tensor_tensor(
                                kb[:].rearrange("p g d -> p (g d)"),
                                ps[:, :ow],
                                bkv_bc[:, o0:o0 + ow], AL.add)
                            st = statsp.tile([128, G, 6], f32, tag="st_k")
                            for g2 in range(G):
                                nc.vector.bn_stats(st[:, g2, :],
                                                   kb[:, g2, :])
                            mu = statsp.tile([128, G], f32, tag="mu_k")
                            rdk = statsp.tile([128, G], f32, tag="rd_k")
                            # combine/apply on Pool: it is idle during the
                            # prologue and this keeps DVE off the critical path
                            emit_ln_stats(st, G, mu, rdk,
                                          1.0 / float(D), EPS, eng=nc.gpsimd)
                            nc.gpsimd.tensor_tensor(
                                kb[:], kb[:],
                                mu[:, :, None].to_broadcast([128, G, D]),
                                AL.subtract)
                            nc.gpsimd.tensor_tensor(
                                kln[t][:, g0:g0 + G, 0:D], kb[:],
                                rdk[:, :, None].to_broadcast([128, G, D]),
                                AL.mult)
                        else:
                            nc.vector.tensor_tensor(
                                v_sb[:, t, o0 - CQ:o0 - CQ + ow],
                                ps[:, :ow],
                                bkv_bc[:, CQ + (o0 - CQ):CQ + (o0 - CQ) + ow],
                                AL.add)

                def kv_tiles():
                    for t in range(2):
                        nc.gpsimd.memset(kln[t][:, :, D:DP], 0.0)
                    return [lambda wi=wi: kv_tile(wi)
                            for wi in range(len(k_tiles + v_tiles))]

                def kv_finish():
                    for t in range(2):
                        nc.sync.dma_start_transpose(
                            kT[:, :, t * 128:(t + 1) * 128],
                            kln[t][:].rearrange("p h d -> p (h d)"))

                def weave(Qs, Ss, Ps):
                    """Interleave PE tile closures: scores spaced out so exp
                    evacuation never stalls the PE; P (PV/out-proj) fills
                    between; Q groups carry the bulk."""
                    seq = []
                    qs, ss, ps = list(Qs), list(Ss), list(Ps)
                    n = max(len(ss), 1)
                    for k in range(n):
                        if ss:
                            seq.append(ss.pop(0))
                        if k % 2 == 0 and qs:
                            seq.append(qs.pop(0))
                        if ps:
                            seq.append(ps.pop(0))
                    # leftovers (no-scores iterations)
                    rest = qs + ps
                    seq.extend(rest)
                    for f in seq:
                        f()

                # ---------- schedule ----------
                emit_load(0)
                if rep == 0:
                    load_wq()
                emit_load(1)
                load_kv_consts()
                q0, q0fin = qproj_tiles(0)
                kvs = kv_tiles()
                prefetch_kv(2)
                weave(q0, [], kvs[:4])
                q0fin()
                emit_load(2)
                if rep == 0:
                    # deferred big preloads: needed first at OP0 (iter 6)
                    nc.gpsimd.dma_start(
                        wout_sb[:], wout[:].rearrange("(k p) o -> p k o", p=128))
                    nc.gpsimd.dma_start(bout_bc[:], bass.AP(
                        tensor=bout_ap.tensor, offset=bout_ap.offset,
                        ap=[[0, 128], *bout_ap.ap]))
                q1, q1fin = qproj_tiles(1)
                weave(q1, [], kvs[4:])
                q1fin()
                kv_finish()
                emit_qpost(0)
                emit_qpost(1)
                for si in range(2, NS):
                    if si + 1 < NS:
                        emit_load(si + 1)
                    Qs, qfin = qproj_tiles(si)
                    Ss, sfin = scores_tiles(si - 2)
                    opfin = None
                    if si >= 5 and si % 2 == 1:
                        Ps = pv_tiles(si - 5)
                    elif si >= 6 and si % 2 == 0:
                        Ps, opfin = op_tiles(si - 6)
                    else:
                        Ps = []
                    weave(Qs, Ss, Ps)
                    qfin()
                    emit_qpost(si)
                    sfin()
                    if opfin is not None:
                        opfin()
                # tail
                S14, s14fin = scores_tiles(NS - 2)
                P10, p10fin = op_tiles(NS - 6)
                weave([], S14, P10)
                s14fin()
                p10fin()
                S15, s15fin = scores_tiles(NS - 1)
                weave([], S15, pv_tiles(NS - 4))
                s15fin()
                P12, p12fin = op_tiles(NS - 4)
                weave([], [], P12)
                p12fin()
                weave([], [], pv_tiles(NS - 2))
                P14, p14fin = op_tiles(NS - 2)
                weave([], [], P14)
                p14fin()

            for _rep in range(reps):
                emit_body(_rep)

    nc.finalize()
    return nc


def _prep(inputs):
    """Host-side shared (per-core independent parts built in kernel())."""
    x = np.asarray(inputs['x'], np.float32)
    y = np.asarray(inputs['y'], np.float32)
    cos = np.asarray(inputs['cos'], np.float32)
    sin = np.asarray(inputs['sin'], np.float32)
    Wq = np.asarray(inputs['Wq'], np.float32)
    bq = np.asarray(inputs['bq'], np.float32)
    Wkv = np.asarray(inputs['Wkv'], np.float32)
    bkv = np.asarray(inputs['bkv'], np.float32)
    qn_w = np.asarray(inputs['qn_w'], np.float32)
    qn_b = np.asarray(inputs['qn_b'], np.float32)
    kn_w = np.asarray(inputs['kn_w'], np.float32)
    kn_b = np.asarray(inputs['kn_b'], np.float32)  # noqa: F841  (cancels in softmax)
    Wout = np.asarray(inputs['Wout'], np.float32)
    bout = np.asarray(inputs['bout'], np.float32)

    perm = np.concatenate([np.arange(0, D, 2), np.arange(1, D, 2)])
    swapv = np.concatenate([np.arange(HALF, D), np.arange(0, HALF)])
    sign = np.concatenate([-np.ones(HALF, np.float32), np.ones(HALF, np.float32)])

    # Q weights: permute head_dim within each head, transpose, append bias row
    Wq_p = Wq.reshape(H, D, CQ)[:, perm, :].reshape(CQ, CQ)
    bq_p = bq.reshape(H, D)[:, perm].reshape(CQ)
    wq_ext = np.zeros((128 * KC_Q, CQ), np.float32)
    wq_ext[:CQ] = Wq_p.T
    wq_ext[CQ] = bq_p

    # KV: permute k-half head_dim (bias too), transpose
    Wkv_p = Wkv.reshape(2, H, D, CKV).copy()
    Wkv_p[0] = Wkv_p[0][:, perm, :]
    bkv_p = bkv.reshape(2, H, D).copy()
    bkv_p[0] = bkv_p[0][:, perm]
    wkvT = Wkv_p.reshape(2 * CQ, CKV).T.copy()
    bkv_p = bkv_p.reshape(2 * CQ)

    wq_vec = qn_w[perm]
    wk_vec = kn_w[perm]
    bq_ln = qn_b[perm]

    cos_p = cos[:, perm]
    sin_p = sin[:, perm]
    wfold = wq_vec * wk_vec
    CW = cos_p * wfold[None, :]                                   # [S1, D]
    SW = sign[None, :] * sin_p * (wq_vec[swapv] * wk_vec)[None, :]
    use_badd = bool(np.any(bq_ln != 0.0))
    BA = wk_vec[None, :] * (bq_ln[None, :] * cos_p
                            + sign[None, :] * bq_ln[swapv][None, :] * sin_p)

    return dict(
        x=x, y=y, wq_ext=wq_ext, wkvT=wkvT, bkv_p=bkv_p,
        woutT=Wout.T.copy(), bout=bout, CW=CW, SW=SW, BA=BA,
        use_badd=use_badd)


def _make_in_maps(p):
    use_badd = p['use_badd']
    wq_bf = p['wq_ext'].astype(ml_dtypes.bfloat16)
    wkv_bf = p['wkvT'].astype(ml_dtypes.bfloat16)
    wout_bf = p['woutT'].astype(ml_dtypes.bfloat16)
    in_maps = []
    for c in range(NCORES):
        b = c // 2
        s0 = (c % 2) * S
        xTe = np.zeros((128 * KC_Q, S), np.float32)
        xTe[:CQ] = p['x'][b, s0:s0 + S].T
        xTe[CQ] = 1.0
        cswp = np.zeros((S, 2, DP), np.float32)
        cswp[:, 0, :D] = p['CW'][s0:s0 + S]
        cswp[:, 1, :D] = p['SW'][s0:s0 + S]
        cswp = cswp.astype(ml_dtypes.bfloat16)
        m = {
            'xT': xTe.astype(ml_dtypes.bfloat16),
            'yT': p['y'][b].T.astype(ml_dtypes.bfloat16).copy(),
            'wq': wq_bf, 'wkv': wkv_bf, 'wout': wout_bf,
            'bkv': p['bkv_p'].astype(ml_dtypes.bfloat16),
            'bout': p['bout'].astype(ml_dtypes.bfloat16),
            'csw': cswp,
        }
        if use_badd:
            bap = np.zeros((S, DP), np.float32)
            bap[:, :D] = p['BA'][s0:s0 + S]
            m['badd'] = bap
        in_maps.append(m)
    return in_maps


def get_nc(use_badd, reps=1):
    key = (use_badd, reps)
    if key not in _BUILD_CACHE:
        _BUILD_CACHE[key] = _build(use_badd, reps)
    return _BUILD_CACHE[key]


def kernel(**inputs) -> np.ndarray:
    p = _prep(inputs)
    in_maps = _make_in_maps(p)
    nc = get_nc(p['use_badd'])
    res = run_bass_kernel_spmd(nc, in_maps, core_ids=list(range(NCORES)))
    outp = np.empty((B, S1, CQ), np.float32)
    for c in range(NCORES):
        b = c // 2
        s0 = (c % 2) * S
        outp[b, s0:s0 + S] = np.asarray(res.results[c]['out'],
                                        dtype=np.float32)
    return outp
